# revision 1
# baseline (speedup 1.0000x reference)
"""LSTMCell forward on 8 Trainium2 NeuronCores (Bass/Tile, SPMD data-parallel).

Strategy:
  - Shard the batch (32768) across 8 cores: 4096 rows each.
  - Host-side prep: xh = concat(x, h, axis=1) transposed to [1024, 4096] per
    core so the contraction dim lands on SBUF partitions (no on-device
    transposes); W = vstack(Wx, Wh) [1024, 2048]; bias = bx + bh broadcast to
    [128, 2048].
  - Per core: z = xh_shard @ W + bias via float32r matmuls (fast fp32 path on
    the PE), accumulated over 8 k-chunks of 128 into PSUM [128, 2048]
    (4 banks) per 128-row sub-chunk.
  - Epilogue: DVE evacuates PSUM fused with the bias add, ACT applies
    sigmoid/tanh, DVE computes C_new = f*C + i*g and h_new = o*tanh(C_new).
  - Tiling: macro-chunks of 512 batch rows (1 MiB DMAs), double-buffered.
"""
import sys
from contextlib import nullcontext

if "/opt/trn_rl_repo" not in sys.path:
    sys.path.insert(0, "/opt/trn_rl_repo")

import numpy as np
import concourse.bass as bass
import concourse.mybir as mybir
from concourse.tile import TileContext
from concourse.bass_utils import run_bass_kernel_spmd

F32 = mybir.dt.float32
F32R = mybir.dt.float32r
AF = mybir.ActivationFunctionType

N_CORES = 8
P = 128
DH = 512
DH4 = 4 * DH            # 2048
K = 1024                # concat(x, h) contraction dim
KT = K // P             # 8 k-chunks
B_FULL = 32768
B_CORE = B_FULL // N_CORES   # 4096
MACRO = 512                  # batch rows per macro-chunk (1 MiB DMA tiles)
NMACRO = B_CORE // MACRO     # 8
NSUB = MACRO // P            # 4


def fanout_multi_waits(nc):
    """This walrus build rejects >1 sync wait per instruction: fan extra
    waits out onto single-wait NoOps on the same (in-order) engine."""
    n = 0
    for f in nc.m.functions:
        for bb in f.blocks:
            new = []
            for inst in bb.instructions:
                si = inst.sync_info
                waits = list(si.on_wait) if si and si.on_wait else []
                if len(waits) > 1:
                    for w in waits[:-1]:
                        nop = mybir.InstNoOp(name=f"waitfan_{n}", ins=[], outs=[])
                        n += 1
                        nop.engine = inst.engine
                        nop.sync_info = mybir.SyncInfo(on_wait=[w], on_update=[])
                        new.append(nop)
                    si.on_wait = [waits[-1]]
                new.append(inst)
            bb.instructions = new
    return n


def build_nc(loop_n=None):
    """Build the per-core program. loop_n wraps the body in a device-side
    For_i repeat (timing probe; outputs unchanged since the body is
    idempotent)."""
    nc = bass.Bass()
    xhT = nc.dram_tensor("xhT", [K, B_CORE], F32R, kind="ExternalInput")
    Cin = nc.dram_tensor("Cin", [B_CORE, DH], F32, kind="ExternalInput")
    W = nc.dram_tensor("W", [K, DH4], F32R, kind="ExternalInput")
    biasb = nc.dram_tensor("biasb", [P, DH4], F32, kind="ExternalInput")
    C_new = nc.dram_tensor("C_new", [B_CORE, DH], F32, kind="ExternalOutput")
    h_new = nc.dram_tensor("h_new", [B_CORE, DH], F32, kind="ExternalOutput")

    xhT_r = xhT[:].rearrange("(kt p) b -> p kt b", p=P)     # [128, 8, B_CORE]
    Cin_r = Cin[:].rearrange("(nb p) d -> p nb d", p=P)     # [128, 32, 512]
    W_r = W[:].rearrange("(kt p) j -> p kt j", p=P)         # [128, 8, 2048]
    Cn_r = C_new[:].rearrange("(nb p) d -> p nb d", p=P)
    Hn_r = h_new[:].rearrange("(nb p) d -> p nb d", p=P)

    with TileContext(nc) as tc:
        with (
            tc.tile_pool(name="const", bufs=1) as const,
            tc.tile_pool(name="io", bufs=2) as io,
            tc.tile_pool(name="work", bufs=2) as work,
            tc.tile_pool(name="psum", bufs=2, space=bass.MemorySpace.PSUM) as psum,
        ):
            w_t = const.tile([P, KT, DH4], F32R)
            nc.sync.dma_start(out=w_t[:], in_=W_r)
            bias_t = const.tile([P, DH4], F32)
            nc.sync.dma_start(out=bias_t[:], in_=biasb[:])

            loop = tc.For_i(0, loop_n, 1) if loop_n else nullcontext()
            with loop:
                for mc in range(NMACRO):
                    xh_t = io.tile([P, KT, MACRO], F32R, tag="xh")
                    nc.sync.dma_start(
                        out=xh_t[:], in_=xhT_r[:, :, mc * MACRO:(mc + 1) * MACRO]
                    )
                    c_t = io.tile([P, NSUB, DH], F32, tag="c")
                    nc.sync.dma_start(
                        out=c_t[:], in_=Cin_r[:, NSUB * mc:NSUB * (mc + 1), :]
                    )
                    cn_t = io.tile([P, NSUB, DH], F32, tag="cn")
                    hn_t = io.tile([P, NSUB, DH], F32, tag="hn")

                    for sub in range(NSUB):
                        zp = psum.tile([P, DH4], F32, tag="zp")
                        for kt in range(KT):
                            lhsT = xh_t[:, kt, sub * P:(sub + 1) * P]
                            for j in range(4):
                                nc.tensor.matmul(
                                    zp[:, j * DH:(j + 1) * DH],
                                    lhsT,
                                    w_t[:, kt, j * DH:(j + 1) * DH],
                                    start=(kt == 0),
                                    stop=(kt == KT - 1),
                                )
                        zb = work.tile([P, DH4], F32, tag="zb")
                        nc.vector.tensor_add(zb[:], zp[:], bias_t[:])
                        # gate order [i, f, o, g]: sigmoid on first 3, tanh on g
                        nc.scalar.activation(
                            zb[:, 0:3 * DH], zb[:, 0:3 * DH], AF.Sigmoid
                        )
                        nc.scalar.activation(zb[:, 3 * DH:], zb[:, 3 * DH:], AF.Tanh)
                        fc = work.tile([P, DH], F32, tag="fc")
                        nc.vector.tensor_mul(fc[:], zb[:, DH:2 * DH], c_t[:, sub, :])
                        ig = work.tile([P, DH], F32, tag="ig")
                        nc.vector.tensor_mul(ig[:], zb[:, 0:DH], zb[:, 3 * DH:])
                        nc.vector.tensor_add(cn_t[:, sub, :], fc[:], ig[:])
                        tch = work.tile([P, DH], F32, tag="tch")
                        nc.scalar.activation(tch[:], cn_t[:, sub, :], AF.Tanh)
                        nc.vector.tensor_mul(
                            hn_t[:, sub, :], zb[:, 2 * DH:3 * DH], tch[:]
                        )

                    nc.sync.dma_start(
                        out=Cn_r[:, NSUB * mc:NSUB * (mc + 1), :], in_=cn_t[:]
                    )
                    nc.sync.dma_start(
                        out=Hn_r[:, NSUB * mc:NSUB * (mc + 1), :], in_=hn_t[:]
                    )
    fanout_multi_waits(nc)
    return nc


_NC = None


def _get_nc():
    global _NC
    if _NC is None:
        _NC = build_nc()
    return _NC


def make_in_maps(x, C, h, Wx, bx, Wh, bh):
    x = np.asarray(x, dtype=np.float32)
    C = np.asarray(C, dtype=np.float32)
    h = np.asarray(h, dtype=np.float32)
    W = np.concatenate(
        [np.asarray(Wx, np.float32), np.asarray(Wh, np.float32)], axis=0
    )
    bias = np.asarray(bx, np.float32) + np.asarray(bh, np.float32)
    biasb = np.broadcast_to(bias, (P, DH4)).copy()
    in_maps = []
    for c in range(N_CORES):
        sl = slice(c * B_CORE, (c + 1) * B_CORE)
        xh = np.concatenate([x[sl], h[sl]], axis=1)         # [4096, 1024]
        in_maps.append(
            {
                "xhT": np.ascontiguousarray(xh.T),          # [1024, 4096]
                "Cin": np.ascontiguousarray(C[sl]),
                "W": W,
                "biasb": biasb,
            }
        )
    return in_maps


def kernel(x, C, h, Wx, bx, Wh, bh):
    nc = _get_nc()
    in_maps = make_in_maps(x, C, h, Wx, bx, Wh, bh)
    res = run_bass_kernel_spmd(nc, in_maps, list(range(N_CORES)))
    C_new = np.concatenate([res.results[c]["C_new"] for c in range(N_CORES)], axis=0)
    h_new = np.concatenate([res.results[c]["h_new"] for c in range(N_CORES)], axis=0)
    return (C_new, h_new)



# revision 2
# speedup vs baseline: 1.0040x; 1.0040x over previous
"""LSTMCell forward on 8 Trainium2 NeuronCores (Bass/Tile, SPMD data-parallel).

Strategy (v2):
  - Shard the batch (32768) across 8 cores: 4096 rows each.
  - All IO in fp16 (rel err ~8e-4, tolerance 2e-2): halves HBM traffic vs
    fp32 and doubles DVE throughput.
  - Transposed-z layout: compute z^T [2048 gate dims, batch] so the gate dim
    lands on PSUM partitions. Stationary operand = W 128x128 blocks, moving
    operand = xh^T batch columns. The per-gate bias is then per-partition, so
    ACT fuses PSUM-evacuation + bias + sigmoid/tanh into ONE pass (no DVE
    bias add).
  - Per output block jb (16 of them: gates i,f,o,g x 4 dh-blocks): accumulate
    8 k-chunks into one PSUM bank [128, 512], ACT-evacuate to SBUF fp16.
  - jb order is gate-major per dh-block (i,f,o,g of block d together) so the
    elementwise epilogue for block d (C_new = f*C + i*g; h_new = o*tanh)
    runs on DVE/ACT while the PE streams block d+1.
  - Batch tiled in macro-chunks of 512 columns, double-buffered DMA.
"""
import sys
from contextlib import nullcontext

if "/opt/trn_rl_repo" not in sys.path:
    sys.path.insert(0, "/opt/trn_rl_repo")

import numpy as np
import concourse.bass as bass
import concourse.mybir as mybir
from concourse.tile import TileContext
from concourse.bass_utils import run_bass_kernel_spmd

F32 = mybir.dt.float32
F16 = mybir.dt.float16
AF = mybir.ActivationFunctionType

N_CORES = 8
P = 128
DH = 512
DH4 = 4 * DH            # 2048
K = 1024                # concat(x, h) contraction dim
KT = K // P             # 8 k-chunks
NJ = DH4 // P           # 16 output column blocks of z^T
ND = DH // P            # 4 dh blocks
B_FULL = 32768
B_CORE = B_FULL // N_CORES   # 4096
BM = 512                     # batch columns per macro-chunk (= PSUM bank)
NMACRO = B_CORE // BM        # 8


def fanout_multi_waits(nc):
    """This walrus build rejects >1 sync wait per instruction: fan extra
    waits out onto single-wait NoOps on the same (in-order) engine."""
    n = 0
    for f in nc.m.functions:
        for bb in f.blocks:
            new = []
            for inst in bb.instructions:
                si = inst.sync_info
                waits = list(si.on_wait) if si and si.on_wait else []
                if len(waits) > 1:
                    for w in waits[:-1]:
                        nop = mybir.InstNoOp(name=f"waitfan_{n}", ins=[], outs=[])
                        n += 1
                        nop.engine = inst.engine
                        nop.sync_info = mybir.SyncInfo(on_wait=[w], on_update=[])
                        new.append(nop)
                    si.on_wait = [waits[-1]]
                new.append(inst)
            bb.instructions = new
    return n


def build_nc(loop_n=None):
    """Build the per-core program. loop_n wraps the body in a device-side
    For_i repeat (timing probe; outputs unchanged since the body is
    idempotent)."""
    nc = bass.Bass()
    xhT = nc.dram_tensor("xhT", [K, B_CORE], F16, kind="ExternalInput")
    CT = nc.dram_tensor("CT", [DH, B_CORE], F16, kind="ExternalInput")
    W = nc.dram_tensor("W", [K, DH4], F16, kind="ExternalInput")
    biasT = nc.dram_tensor("biasT", [P, NJ], F32, kind="ExternalInput")
    CnT = nc.dram_tensor("CnT", [DH, B_CORE], F16, kind="ExternalOutput")
    HnT = nc.dram_tensor("HnT", [DH, B_CORE], F16, kind="ExternalOutput")

    xhT_r = xhT[:].rearrange("(kt p) b -> p kt b", p=P)     # [128, 8, 4096]
    W_r = W[:].rearrange("(kt p) j -> p kt j", p=P)         # [128, 8, 2048]
    CT_r = CT[:].rearrange("(d p) b -> p d b", p=P)         # [128, 4, 4096]
    Cn_r = CnT[:].rearrange("(d p) b -> p d b", p=P)
    Hn_r = HnT[:].rearrange("(d p) b -> p d b", p=P)

    with TileContext(nc) as tc:
        with (
            tc.tile_pool(name="const", bufs=1) as const,
            tc.tile_pool(name="io", bufs=2) as io,
            tc.tile_pool(name="work", bufs=2) as work,
            tc.tile_pool(name="psum", bufs=4, space=bass.MemorySpace.PSUM) as psum,
        ):
            w_t = const.tile([P, KT, DH4], F16)
            nc.sync.dma_start(out=w_t[:], in_=W_r)
            bias_t = const.tile([P, NJ], F32)
            nc.sync.dma_start(out=bias_t[:], in_=biasT[:])

            loop = tc.For_i(0, loop_n, 1) if loop_n else nullcontext()
            with loop:
                for mc in range(NMACRO):
                    sl = slice(mc * BM, (mc + 1) * BM)
                    xh_t = io.tile([P, KT, BM], F16, tag="xh")
                    nc.sync.dma_start(out=xh_t[:], in_=xhT_r[:, :, sl])
                    c_t = io.tile([P, ND, BM], F16, tag="c")
                    nc.sync.dma_start(out=c_t[:], in_=CT_r[:, :, sl])
                    g_t = work.tile([P, NJ, BM], F16, tag="gates")
                    cn_t = io.tile([P, ND, BM], F16, tag="cn")
                    hn_t = io.tile([P, ND, BM], F16, tag="hn")

                    for d in range(ND):
                        # gates i, f, o, g for dh-block d -> z^T column
                        # blocks d, 4+d, 8+d, 12+d
                        for gi in range(4):
                            jb = gi * ND + d
                            zp = psum.tile([P, BM], F32, tag="zp")
                            for kt in range(KT):
                                nc.tensor.matmul(
                                    zp[:],
                                    w_t[:, kt, jb * P:(jb + 1) * P],
                                    xh_t[:, kt, :],
                                    start=(kt == 0),
                                    stop=(kt == KT - 1),
                                )
                            nc.scalar.activation(
                                g_t[:, jb, :], zp[:],
                                AF.Tanh if gi == 3 else AF.Sigmoid,
                                bias=bias_t[:, jb:jb + 1],
                            )
                        fc = work.tile([P, BM], F16, tag="fc")
                        nc.vector.tensor_mul(fc[:], g_t[:, ND + d, :], c_t[:, d, :])
                        ig = work.tile([P, BM], F16, tag="ig")
                        nc.vector.tensor_mul(ig[:], g_t[:, d, :], g_t[:, 3 * ND + d, :])
                        nc.vector.tensor_add(cn_t[:, d, :], fc[:], ig[:])
                        th = work.tile([P, BM], F16, tag="th")
                        nc.scalar.activation(th[:], cn_t[:, d, :], AF.Tanh)
                        nc.vector.tensor_mul(hn_t[:, d, :], g_t[:, 2 * ND + d, :], th[:])

                    nc.sync.dma_start(out=Cn_r[:, :, sl], in_=cn_t[:])
                    nc.sync.dma_start(out=Hn_r[:, :, sl], in_=hn_t[:])
    fanout_multi_waits(nc)
    return nc


_NC = None


def _get_nc():
    global _NC
    if _NC is None:
        _NC = build_nc()
    return _NC


def make_in_maps(x, C, h, Wx, bx, Wh, bh):
    x = np.asarray(x, dtype=np.float32)
    C = np.asarray(C, dtype=np.float32)
    h = np.asarray(h, dtype=np.float32)
    W16 = np.concatenate(
        [np.asarray(Wx, np.float32), np.asarray(Wh, np.float32)], axis=0
    ).astype(np.float16)                                    # [1024, 2048]
    bias = np.asarray(bx, np.float32) + np.asarray(bh, np.float32)
    biasT = np.ascontiguousarray(bias.reshape(NJ, P).T)     # [128, 16]
    in_maps = []
    for c in range(N_CORES):
        sl = slice(c * B_CORE, (c + 1) * B_CORE)
        xhT = np.empty((K, B_CORE), dtype=np.float16)
        xhT[:DH] = x[sl].T
        xhT[DH:] = h[sl].T
        CT = np.ascontiguousarray(C[sl].T.astype(np.float16))
        in_maps.append({"xhT": xhT, "CT": CT, "W": W16, "biasT": biasT})
    return in_maps


def kernel(x, C, h, Wx, bx, Wh, bh):
    nc = _get_nc()
    in_maps = make_in_maps(x, C, h, Wx, bx, Wh, bh)
    res = run_bass_kernel_spmd(nc, in_maps, list(range(N_CORES)))
    C_new = np.concatenate(
        [res.results[c]["CnT"].astype(np.float32).T for c in range(N_CORES)], axis=0
    )
    h_new = np.concatenate(
        [res.results[c]["HnT"].astype(np.float32).T for c in range(N_CORES)], axis=0
    )
    return (C_new, h_new)
